# revision 35
# baseline (speedup 1.0000x reference)
"""Trainium2 Bass kernel for 16-head MultiHeadAttention (B=2, S=2048, D=1024).

Sharding: 8 cores = 2 (batch) x 4 (head groups of 4 heads).  Each core
computes, for its batch b and head group g:
  Q_g = x_q @ Wq[:, g] ; K_g, V_g likewise
  ctx_g = softmax(Q_g K_g^T / sqrt(64)) V_g            (4 heads)
  out_partial = ctx_g @ Wo[g, :]                        [2048, 1024]
Host sums the 4 partials per batch and adds bo.

On-device layout notes:
  - activations are fed transposed (features on partitions) so every matmul
    contracts over the partition dim without any on-device transposes
  - scores are computed transposed (s^T[keys, queries]) so the exp'd
    probabilities feed the ctx matmul directly
  - softmax skips max-subtraction (scores ~ N(0,1) by construction; fp32 exp
    is exact to ~6 sigma) and gets denominators from a ones-column appended
    to V (free: matmul time only depends on the moving free dim)
"""

import os
import sys

sys.path.insert(0, "/opt/trn_rl_repo")

import numpy as np

import concourse.bass as bass
import concourse.tile as tile
from concourse import bacc, mybir
from concourse.bass_utils import run_bass_kernel_spmd

F32 = mybir.dt.float32
F16 = mybir.dt.float16
AF = mybir.ActivationFunctionType

D = 1024          # model dim
S = 2048          # sequence length (per batch)
HPC = 4           # heads per core
DK = 64           # head dim
HC = HPC * DK     # head cols per core = 256
FC = 8            # feature chunks of 128 (contraction for projections)
TT = 4            # token tiles of 512
KC = 16           # key chunks of 128

LAST_RESULTS = None  # BassKernelResults of the most recent run (for test.py)
_NC_CACHE = None


def build_nc():
    # Bacc (not raw Bass): its compile() runs move_matmul_waits_to_ldweights
    # and generate_event_semaphores, which legalize multi-semaphore waits
    # down to the 1-wait-per-instruction hardware limit.
    nc = bacc.Bacc("TRN2", target_bir_lowering=False, debug=False)

    xq = nc.dram_tensor("xq_t", [D, S], F32, kind="ExternalInput")
    xk = nc.dram_tensor("xk_t", [D, S], F32, kind="ExternalInput")
    xv = nc.dram_tensor("xv_t", [D, S], F32, kind="ExternalInput")
    wq = nc.dram_tensor("wq", [D, HC], F32, kind="ExternalInput")
    wk = nc.dram_tensor("wk", [D, HC], F32, kind="ExternalInput")
    wv = nc.dram_tensor("wv", [D, HC], F32, kind="ExternalInput")
    wo = nc.dram_tensor("wo", [HC, D], F32, kind="ExternalInput")
    bq = nc.dram_tensor("bq2", [128, 2], F32, kind="ExternalInput")
    bk = nc.dram_tensor("bk2", [128, 2], F32, kind="ExternalInput")
    bv = nc.dram_tensor("bv_bc", [128, HC], F32, kind="ExternalInput")
    out_p = nc.dram_tensor("out_p", [S, D], F32, kind="ExternalOutput")

    with tile.TileContext(nc) as tc:
        _emit(tc, xq, xk, xv, wq, wk, wv, wo, bq, bk, bv, out_p)
    nc.compile()
    return nc


def _emit(tc, xq, xk, xv, wq, wk, wv, wo, bq, bk, bv, out_p):
    nc = tc.nc

    with (
        nc.allow_low_precision(
            reason="fp16 matmul operands; all magnitudes well within fp16 range"
        ),
        tc.tile_pool(name="const", bufs=1) as cpool,
        tc.tile_pool(name="big", bufs=1) as bigpool,
        tc.tile_pool(name="xin", bufs=3) as xin,
        tc.tile_pool(name="pT", bufs=3) as ptpool,
        tc.tile_pool(name="rc", bufs=2) as rcpool,
        tc.tile_pool(name="osb", bufs=3) as osb,
    ):
        # ---- resident weights / biases ----
        wq_sb = cpool.tile([128, FC, HC], F16, tag="wq")
        wk_sb = cpool.tile([128, FC, HC], F16, tag="wk")
        wv_sb = cpool.tile([128, FC, HC], F16, tag="wv")
        wo_sb = cpool.tile([128, 2, D], F16, tag="wo")
        bq_sb = cpool.tile([128, 2], F32, tag="bq")
        bk_sb = cpool.tile([128, 2], F32, tag="bk")
        bv_sb = cpool.tile([128, HC], F32, tag="bv")
        ones_sb = cpool.tile([1, DK], F16, tag="ones")

        nc.gpsimd.dma_start(wq_sb[:], wq[:].rearrange("(a p) c -> p a c", p=128))
        nc.gpsimd.dma_start(wk_sb[:], wk[:].rearrange("(a p) c -> p a c", p=128))
        nc.gpsimd.dma_start(wv_sb[:], wv[:].rearrange("(a p) c -> p a c", p=128))
        nc.gpsimd.dma_start(wo_sb[:], wo[:].rearrange("(a p) c -> p a c", p=128))
        nc.sync.dma_start(bq_sb[:], bq[:])
        nc.sync.dma_start(bk_sb[:], bk[:])
        nc.sync.dma_start(bv_sb[:], bv[:])
        # memset can't write f32r; memset f32 scratch then copy-cast
        ones_f32 = cpool.tile([128, DK], F32, tag="ones_f32")
        nc.vector.memset(ones_f32[:], 1.0)
        nc.vector.tensor_copy(ones_sb[:], ones_f32[0:1, :])

        # ---- resident activations ----
        kT_sb = bigpool.tile([128, 2, S], F16, tag="kT")        # K^T (2 m-tiles)
        v_sb = bigpool.tile([128, HPC, KC, DK + 1], F16, tag="v")  # V natural +1s
        qT_sb = [
            bigpool.tile([128, 2, 512], F16, tag=f"qT{t}", name=f"qT{t}")
            for t in range(TT)
        ]
        cT_sb = [
            bigpool.tile([128, 2, 512], F16, tag=f"cT{t}", name=f"cT{t}")
            for t in range(TT)
        ]

        for h in range(HPC):
            nc.vector.tensor_copy(
                v_sb[:, h, :, DK : DK + 1],
                ones_f32[:, 0:KC].rearrange("p (f o) -> p f o", o=1),
            )

        # ---- projections ----
        # NOTE on semaphore waits: this toolchain's walrus rejects matmuls
        # with more than ONE semaphore wait.  Tile emits a wait only when the
        # engine has not yet observed that semaphore tick, so we insert tiny
        # "absorber" matmuls (1x1x1 into a dummy PSUM bank, no readers) whose
        # sole job is to make the PE observe a fresh DMA semaphore before a
        # real matmul needs it.  PSUM->SBUF projection copies run on ACT so
        # that psum-slot-release waits ride the ACT semaphore, never adding a
        # second distinct semaphore to any matmul.

        def absorb(src_ap):
            # standalone LDWEIGHTS (bf16 view; fp32 LDW is rejected) — a
            # ~50ns PE instruction whose only effect is waiting on src's
            # producer semaphore; the real matmuls are self-loading.
            v = src_ap.bitcast(mybir.dt.bfloat16)
            nc.tensor.ldweights(weights=v[0:1, 0:64])

        def load_x(x_dram, t):
            # all 8 feature chunks of one token tile in a single cast-DMA
            xt = xin.tile([128, FC, 512], F16, tag="xin")
            nc.gpsimd.dma_start(
                xt[:],
                x_dram[:].rearrange("(a p) s -> p a s", p=128)[
                    :, :, t * 512 : (t + 1) * 512
                ],
            )
            absorb(xt[:, 0, :])
            return xt

        def proj_T(x_dram, w_sb, b_sb, dst):
            # dst(mt, tt) -> AP [128, 512]; computes (x @ W)^T + b
            for t in range(TT):
                xt = load_x(x_dram, t)
                for mt in range(2):
                    ps = psP.tile([128, 512], F32, tag="psP")
                    for f in range(FC):
                        nc.tensor.matmul(
                            ps[:],
                            lhsT=w_sb[:, f, mt * 128 : (mt + 1) * 128],
                            rhs=xt[:, f, :],
                            start=(f == 0),
                            stop=(f == FC - 1),
                        )
                    nc.scalar.add(dst(mt, t), ps[:], b_sb[:, mt : mt + 1])

        with tc.tile_pool(name="psP", bufs=4, space="PSUM") as psP:
            for w in (wk_sb, wv_sb, wq_sb, wo_sb):
                absorb(w[:, 0, :])
            # ACT reads biases during projection copies and DVE reads bv
            # during the V adds; pre-observe those DMA semaphores here so
            # those instructions carry only the PE wait.
            scr = cpool.tile([1, 4], F32, tag="scr")
            nc.scalar.copy(scr[0:1, 0:1], bq_sb[0:1, 0:1])
            nc.scalar.copy(scr[0:1, 1:2], bk_sb[0:1, 0:1])
            nc.vector.tensor_copy(scr[0:1, 2:3], bv_sb[0:1, 0:1])

            proj_T(xk, wk_sb, bk_sb, lambda mt, t: kT_sb[:, mt, t * 512 : (t + 1) * 512])

            # V in natural layout [tokens, cols], with bias broadcast tensor
            for t in range(TT):
                xt = load_x(xv, t)
                for j in range(4):
                    kt = t * 4 + j
                    ps = psP.tile([128, HC], F32, tag="vps")
                    for f in range(FC):
                        nc.tensor.matmul(
                            ps[:],
                            lhsT=xt[:, f, j * 128 : (j + 1) * 128],
                            rhs=wv_sb[:, f, :],
                            start=(f == 0),
                            stop=(f == FC - 1),
                        )
                    nc.vector.tensor_add(
                        v_sb[:, :, kt, 0:DK],
                        ps[:].rearrange("p (h c) -> p h c", h=HPC),
                        bv_sb[:].rearrange("p (h c) -> p h c", h=HPC),
                    )

            proj_T(xq, wq_sb, bq_sb, lambda mt, t: qT_sb[t][:, mt, :])

        # ---- attention ----
        GRP = 2  # key-chunks per exp call; A0/A1 = 2 banks each
        with (
            tc.tile_pool(name="psA", bufs=1, space="PSUM") as psA,
            tc.tile_pool(name="psC", bufs=1, space="PSUM") as psC,
            tc.tile_pool(name="psO", bufs=1, space="PSUM") as psO,
        ):
            # make PE observe the last projection ACT tick and last V DVE
            # tick before attention matmuls need them
            absorb(qT_sb[TT - 1][:, 1, :])
            absorb(v_sb[:, HPC - 1, KC - 1, :])
            last_cU = None
            last_P = None
            for qt in range(TT):
                for hp in range(2):  # head pairs (2hp, 2hp+1); mt == hp
                    h0 = 2 * hp
                    Cs = [
                        psC.tile([DK + 1, 512], F32, tag="C0", name="C0"),
                        psC.tile([DK + 1, 512], F32, tag="C1", name="C1"),
                    ]
                    for g0 in range(0, KC, GRP):
                        kcs = range(g0, min(g0 + GRP, KC))
                        w = len(kcs) * 512
                        # per-head A tiles; the adjacent row-packed score
                        # matmuls (rows 0:64 / 64:128 via lhsT base_partition)
                        # run concurrently in disjoint PE row groups
                        As = [
                            psA.tile([128, GRP, 512], F32, tag="A0", name="A0"),
                            psA.tile([128, GRP, 512], F32, tag="A1", name="A1"),
                        ]
                        # HAM heater: one full-array (K=128, M=128) matmul
                        # into the bank the next score MM overwrites anyway;
                        # keeps the PE activity monitor at K=8/8 through the
                        # half-utilization score/ctx mix.
                        nc.tensor.matmul(
                            As[0][:, 0, :],
                            lhsT=kT_sb[:, hp, 0:128],
                            rhs=kT_sb[:, hp, 0:512],
                            start=True,
                            stop=True,
                        )
                        for j, kc in enumerate(kcs):
                            for i in range(2):
                                p0 = i * 64
                                nc.tensor.matmul(
                                    As[i][:, j, :],
                                    lhsT=kT_sb[p0 : p0 + 64, hp, kc * 128 : (kc + 1) * 128],
                                    rhs=qT_sb[qt][p0 : p0 + 64, hp, :],
                                    start=True,
                                    stop=True,
                                )
                        Ps = [
                            ptpool.tile([128, GRP, 512], F16, tag="pT0", name="P0"),
                            ptpool.tile([128, GRP, 512], F16, tag="pT1", name="P1"),
                        ]
                        last_P = Ps[1]
                        for i in range(2):
                            nc.scalar.activation(
                                Ps[i][:].rearrange("p a b -> p (a b)")[:, :w],
                                As[i][:].rearrange("p a b -> p (a b)")[:, :w],
                                AF.Exp,
                                scale=0.125,
                            )
                        for j, kc in enumerate(kcs):
                            for i in range(2):
                                nc.tensor.matmul(
                                    Cs[i][:],
                                    lhsT=v_sb[:, h0 + i, kc, :],
                                    rhs=Ps[i][:, j, :],
                                    start=(kc == 0),
                                    stop=(kc == KC - 1),
                                )
                    for i in range(2):
                        p0 = i * 64
                        # bufs=16: one slot per (h, qt) so this copy never
                        # carries a foreign slot-release wait
                        cU = rcpool.tile([DK + 1, 512], F32, tag="cU", bufs=16)
                        last_cU = cU
                        nc.vector.tensor_copy(cU[:], Cs[i][:])
                        rc = rcpool.tile([1, 512], F32, tag="rc")
                        nc.vector.reciprocal(rc[:], cU[DK : DK + 1, :])
                        Sb = rcpool.tile([DK, 512], F32, tag="Sbc")
                        nc.gpsimd.partition_broadcast(Sb[:], rc[:])
                        nc.vector.tensor_mul(
                            cT_sb[qt][p0 : p0 + 64, hp, :], cU[0:DK, :], Sb[:]
                        )

                # output projection for this qt; its full-array matmuls
                # interleave with the next qt's attention and keep the PE
                # activity monitor warm (partial; host sums head groups)
                for j in range(4):
                    t16 = qt * 4 + j
                    o_ps = psO.tile([128, D], F32, tag="o")
                    for nb in range(2):
                        for c2 in range(2):
                            nc.tensor.matmul(
                                o_ps[:, nb * 512 : (nb + 1) * 512],
                                lhsT=cT_sb[qt][:, c2, j * 128 : (j + 1) * 128],
                                rhs=wo_sb[:, c2, nb * 512 : (nb + 1) * 512],
                                start=(c2 == 0),
                                stop=(c2 == 1),
                            )
                    ob = osb.tile([128, D], F32, tag="ob")
                    nc.vector.tensor_copy(ob[:], o_ps[:])
                    nc.sync.dma_start(out_p[t16 * 128 : (t16 + 1) * 128, :], ob[:])


def _shard_inputs(query, key_in, value, Wq, bq, Wk, bk, Wv, bv, Wo, bo):
    q = np.ascontiguousarray(np.asarray(query, dtype=np.float32))
    k = np.ascontiguousarray(np.asarray(key_in, dtype=np.float32))
    v = np.ascontiguousarray(np.asarray(value, dtype=np.float32))
    Wq, Wk, Wv, Wo = (np.asarray(a, np.float32) for a in (Wq, Wk, Wv, Wo))
    bq, bk, bv = (np.asarray(a, np.float32) for a in (bq, bk, bv))

    in_maps = []
    for core in range(8):
        b, g = divmod(core, 4)
        sl = slice(g * HC, (g + 1) * HC)
        in_maps.append(
            {
                "xq_t": np.ascontiguousarray(q[b].T),
                "xk_t": np.ascontiguousarray(k[b].T),
                "xv_t": np.ascontiguousarray(v[b].T),
                "wq": np.ascontiguousarray(Wq[:, sl]),
                "wk": np.ascontiguousarray(Wk[:, sl]),
                "wv": np.ascontiguousarray(Wv[:, sl]),
                "wo": np.ascontiguousarray(Wo[sl, :]),
                "bq2": np.ascontiguousarray(bq[sl].reshape(2, 128).T),
                "bk2": np.ascontiguousarray(bk[sl].reshape(2, 128).T),
                "bv_bc": np.ascontiguousarray(
                    np.broadcast_to(bv[sl], (128, HC))
                ),
            }
        )
    return in_maps


def kernel(query, key_in, value, Wq, bq, Wk, bk, Wv, bv, Wo, bo):
    global LAST_RESULTS, _NC_CACHE
    if _NC_CACHE is None:
        _NC_CACHE = build_nc()
    nc = _NC_CACHE

    in_maps = _shard_inputs(query, key_in, value, Wq, bq, Wk, bk, Wv, bv, Wo, bo)
    trace = bool(os.environ.get("BASS_TRACE"))
    res = run_bass_kernel_spmd(nc, in_maps, core_ids=list(range(8)), trace=trace)
    LAST_RESULTS = res

    bo = np.asarray(bo, np.float32)
    out = np.empty((2, S, D), dtype=np.float32)
    for b in range(2):
        acc = res.results[4 * b]["out_p"].astype(np.float32)
        for g in range(1, 4):
            acc = acc + res.results[4 * b + g]["out_p"]
        out[b] = acc + bo
    return out


# revision 40
# speedup vs baseline: 1.0083x; 1.0083x over previous
"""Trainium2 Bass kernel for 16-head MultiHeadAttention (B=2, S=2048, D=1024).

Sharding: 8 cores = 2 (batch) x 4 (head groups of 4 heads).  Each core
computes, for its batch b and head group g:
  Q_g = x_q @ Wq[:, g] ; K_g, V_g likewise
  ctx_g = softmax(Q_g K_g^T / sqrt(64)) V_g            (4 heads)
  out_partial = ctx_g @ Wo[g, :]                        [2048, 1024]
Host sums the 4 partials per batch and adds bo.

On-device layout notes:
  - activations are fed transposed (features on partitions) so every matmul
    contracts over the partition dim without any on-device transposes
  - scores are computed transposed (s^T[keys, queries]) so the exp'd
    probabilities feed the ctx matmul directly
  - softmax skips max-subtraction (scores ~ N(0,1) by construction; fp32 exp
    is exact to ~6 sigma) and gets denominators from a ones-column appended
    to V (free: matmul time only depends on the moving free dim)
"""

import os
import sys

sys.path.insert(0, "/opt/trn_rl_repo")

import numpy as np

import concourse.bass as bass
import concourse.tile as tile
from concourse import bacc, mybir
from concourse.bass_utils import run_bass_kernel_spmd

F32 = mybir.dt.float32
F16 = mybir.dt.float16
AF = mybir.ActivationFunctionType

D = 1024          # model dim
S = 2048          # sequence length (per batch)
HPC = 4           # heads per core
DK = 64           # head dim
HC = HPC * DK     # head cols per core = 256
FC = 8            # feature chunks of 128 (contraction for projections)
TT = 4            # token tiles of 512
KC = 16           # key chunks of 128

LAST_RESULTS = None  # BassKernelResults of the most recent run (for test.py)
_NC_CACHE = None


def build_nc():
    # Bacc (not raw Bass): its compile() runs move_matmul_waits_to_ldweights
    # and generate_event_semaphores, which legalize multi-semaphore waits
    # down to the 1-wait-per-instruction hardware limit.
    nc = bacc.Bacc("TRN2", target_bir_lowering=False, debug=False)

    xq = nc.dram_tensor("xq_t", [D, S], F32, kind="ExternalInput")
    xk = nc.dram_tensor("xk_t", [D, S], F32, kind="ExternalInput")
    xv = nc.dram_tensor("xv_t", [D, S], F32, kind="ExternalInput")
    wq = nc.dram_tensor("wq", [D, HC], F32, kind="ExternalInput")
    wk = nc.dram_tensor("wk", [D, HC], F32, kind="ExternalInput")
    wv = nc.dram_tensor("wv", [D, HC], F32, kind="ExternalInput")
    wo = nc.dram_tensor("wo", [HC, D], F32, kind="ExternalInput")
    bq = nc.dram_tensor("bq2", [128, 2], F32, kind="ExternalInput")
    bk = nc.dram_tensor("bk2", [128, 2], F32, kind="ExternalInput")
    bv = nc.dram_tensor("bv_bc", [128, HC], F32, kind="ExternalInput")
    out_p = nc.dram_tensor("out_p", [S, D], F32, kind="ExternalOutput")

    with tile.TileContext(nc) as tc:
        _emit(tc, xq, xk, xv, wq, wk, wv, wo, bq, bk, bv, out_p)
    nc.compile()
    return nc


def _emit(tc, xq, xk, xv, wq, wk, wv, wo, bq, bk, bv, out_p):
    nc = tc.nc

    with (
        nc.allow_low_precision(
            reason="fp16 matmul operands; all magnitudes well within fp16 range"
        ),
        tc.tile_pool(name="const", bufs=1) as cpool,
        tc.tile_pool(name="big", bufs=1) as bigpool,
        tc.tile_pool(name="xin", bufs=3) as xin,
        tc.tile_pool(name="pT", bufs=3) as ptpool,
        tc.tile_pool(name="rc", bufs=2) as rcpool,
        tc.tile_pool(name="osb", bufs=3) as osb,
    ):
        # ---- resident weights / biases ----
        wq_sb = cpool.tile([128, FC, HC], F16, tag="wq")
        wk_sb = cpool.tile([128, FC, HC], F16, tag="wk")
        wv_sb = cpool.tile([128, FC, HC], F16, tag="wv")
        wo_sb = cpool.tile([128, 2, D], F16, tag="wo")
        bq_sb = cpool.tile([128, 2], F32, tag="bq")
        bk_sb = cpool.tile([128, 2], F32, tag="bk")
        bv_sb = cpool.tile([128, HC], F32, tag="bv")
        ones_sb = cpool.tile([1, DK], F16, tag="ones")

        nc.gpsimd.dma_start(wq_sb[:], wq[:].rearrange("(a p) c -> p a c", p=128))
        nc.gpsimd.dma_start(wk_sb[:], wk[:].rearrange("(a p) c -> p a c", p=128))
        nc.gpsimd.dma_start(wv_sb[:], wv[:].rearrange("(a p) c -> p a c", p=128))
        nc.gpsimd.dma_start(wo_sb[:], wo[:].rearrange("(a p) c -> p a c", p=128))
        nc.sync.dma_start(bq_sb[:], bq[:])
        nc.sync.dma_start(bk_sb[:], bk[:])
        nc.sync.dma_start(bv_sb[:], bv[:])
        # memset can't write f32r; memset f32 scratch then copy-cast
        ones_f32 = cpool.tile([128, DK], F32, tag="ones_f32")
        nc.vector.memset(ones_f32[:], 1.0)
        nc.vector.tensor_copy(ones_sb[:], ones_f32[0:1, :])

        # ---- resident activations ----
        kT_sb = bigpool.tile([128, 2, S], F16, tag="kT")        # K^T (2 m-tiles)
        v_sb = bigpool.tile([128, HPC, KC, DK + 1], F16, tag="v")  # V natural +1s
        qT_sb = [
            bigpool.tile([128, 2, 512], F16, tag=f"qT{t}", name=f"qT{t}")
            for t in range(TT)
        ]
        cT_sb = [
            bigpool.tile([128, 2, 512], F16, tag=f"cT{t}", name=f"cT{t}")
            for t in range(TT)
        ]

        for h in range(HPC):
            nc.vector.tensor_copy(
                v_sb[:, h, :, DK : DK + 1],
                ones_f32[:, 0:KC].rearrange("p (f o) -> p f o", o=1),
            )

        # ---- projections ----
        # NOTE on semaphore waits: this toolchain's walrus rejects matmuls
        # with more than ONE semaphore wait.  Tile emits a wait only when the
        # engine has not yet observed that semaphore tick, so we insert tiny
        # "absorber" matmuls (1x1x1 into a dummy PSUM bank, no readers) whose
        # sole job is to make the PE observe a fresh DMA semaphore before a
        # real matmul needs it.  PSUM->SBUF projection copies run on ACT so
        # that psum-slot-release waits ride the ACT semaphore, never adding a
        # second distinct semaphore to any matmul.

        def absorb(src_ap):
            # standalone LDWEIGHTS (bf16 view; fp32 LDW is rejected) — a
            # ~50ns PE instruction whose only effect is waiting on src's
            # producer semaphore; the real matmuls are self-loading.
            v = src_ap.bitcast(mybir.dt.bfloat16)
            nc.tensor.ldweights(weights=v[0:1, 0:64])

        def load_x(x_dram, t):
            # all 8 feature chunks of one token tile in a single cast-DMA
            xt = xin.tile([128, FC, 512], F16, tag="xin")
            nc.gpsimd.dma_start(
                xt[:],
                x_dram[:].rearrange("(a p) s -> p a s", p=128)[
                    :, :, t * 512 : (t + 1) * 512
                ],
            )
            absorb(xt[:, 0, :])
            return xt

        def proj_T(x_dram, w_sb, b_sb, dst):
            # dst(mt, tt) -> AP [128, 512]; computes (x @ W)^T + b
            for t in range(TT):
                xt = load_x(x_dram, t)
                for mt in range(2):
                    ps = psP.tile([128, 512], F32, tag="psP")
                    for f in range(FC):
                        nc.tensor.matmul(
                            ps[:],
                            lhsT=w_sb[:, f, mt * 128 : (mt + 1) * 128],
                            rhs=xt[:, f, :],
                            start=(f == 0),
                            stop=(f == FC - 1),
                        )
                    nc.scalar.add(dst(mt, t), ps[:], b_sb[:, mt : mt + 1])

        with tc.tile_pool(name="psP", bufs=4, space="PSUM") as psP:
            for w in (wk_sb, wv_sb, wq_sb, wo_sb):
                absorb(w[:, 0, :])
            # ACT reads biases during projection copies and DVE reads bv
            # during the V adds; pre-observe those DMA semaphores here so
            # those instructions carry only the PE wait.
            scr = cpool.tile([1, 4], F32, tag="scr")
            nc.scalar.copy(scr[0:1, 0:1], bq_sb[0:1, 0:1])
            nc.scalar.copy(scr[0:1, 1:2], bk_sb[0:1, 0:1])
            nc.vector.tensor_copy(scr[0:1, 2:3], bv_sb[0:1, 0:1])

            proj_T(xk, wk_sb, bk_sb, lambda mt, t: kT_sb[:, mt, t * 512 : (t + 1) * 512])

            # V in natural layout [tokens, cols], with bias broadcast tensor
            for t in range(TT):
                xt = load_x(xv, t)
                for j in range(4):
                    kt = t * 4 + j
                    ps = psP.tile([128, HC], F32, tag="vps")
                    for f in range(FC):
                        nc.tensor.matmul(
                            ps[:],
                            lhsT=xt[:, f, j * 128 : (j + 1) * 128],
                            rhs=wv_sb[:, f, :],
                            start=(f == 0),
                            stop=(f == FC - 1),
                        )
                    nc.vector.tensor_add(
                        v_sb[:, :, kt, 0:DK],
                        ps[:].rearrange("p (h c) -> p h c", h=HPC),
                        bv_sb[:].rearrange("p (h c) -> p h c", h=HPC),
                    )

            proj_T(xq, wq_sb, bq_sb, lambda mt, t: qT_sb[t][:, mt, :])

        # ---- attention ----
        GRP = 2  # key-chunks per exp call; A0/A1 = 2 banks each
        with (
            tc.tile_pool(name="psA", bufs=1, space="PSUM") as psA,
            tc.tile_pool(name="psC", bufs=1, space="PSUM") as psC,
            tc.tile_pool(name="psO", bufs=1, space="PSUM") as psO,
        ):
            # make PE observe the last projection ACT tick and last V DVE
            # tick before attention matmuls need them
            absorb(qT_sb[TT - 1][:, 1, :])
            absorb(v_sb[:, HPC - 1, KC - 1, :])
            last_cU = None
            last_P = None
            for qt in range(TT):
                for hp in range(2):  # head pairs (2hp, 2hp+1); mt == hp
                    h0 = 2 * hp
                    Cs = [
                        psC.tile([DK + 1, 512], F32, tag="C0", name="C0"),
                        psC.tile([DK + 1, 512], F32, tag="C1", name="C1"),
                    ]
                    for g0 in range(0, KC, GRP):
                        kcs = range(g0, min(g0 + GRP, KC))
                        w = len(kcs) * 512
                        # per-head A tiles; the adjacent row-packed score
                        # matmuls (rows 0:64 / 64:128 via lhsT base_partition)
                        # run concurrently in disjoint PE row groups
                        As = [
                            psA.tile([128, GRP, 512], F32, tag="A0", name="A0"),
                            psA.tile([128, GRP, 512], F32, tag="A1", name="A1"),
                        ]
                        # HAM heater: one full-array (K=128, M=128) matmul
                        # into the bank the next score MM overwrites anyway;
                        # keeps the PE activity monitor at K=8/8 through the
                        # half-utilization score/ctx mix.
                        nc.tensor.matmul(
                            As[0][:, 0, :],
                            lhsT=kT_sb[:, hp, 0:128],
                            rhs=kT_sb[:, hp, 0:512],
                            start=True,
                            stop=True,
                        )
                        for j, kc in enumerate(kcs):
                            for i in range(2):
                                p0 = i * 64
                                nc.tensor.matmul(
                                    As[i][:, j, :],
                                    lhsT=kT_sb[p0 : p0 + 64, hp, kc * 128 : (kc + 1) * 128],
                                    rhs=qT_sb[qt][p0 : p0 + 64, hp, :],
                                    start=True,
                                    stop=True,
                                )
                        Ps = [
                            ptpool.tile([128, GRP, 512], F16, tag="pT0", name="P0"),
                            ptpool.tile([128, GRP, 512], F16, tag="pT1", name="P1"),
                        ]
                        last_P = Ps[1]
                        for i in range(2):
                            nc.scalar.activation(
                                Ps[i][:].rearrange("p a b -> p (a b)")[:, :w],
                                As[i][:].rearrange("p a b -> p (a b)")[:, :w],
                                AF.Exp,
                                scale=0.125,
                            )
                        for j, kc in enumerate(kcs):
                            for i in range(2):
                                nc.tensor.matmul(
                                    Cs[i][:],
                                    lhsT=v_sb[:, h0 + i, kc, :],
                                    rhs=Ps[i][:, j, :],
                                    start=(kc == 0),
                                    stop=(kc == KC - 1),
                                )
                    for i in range(2):
                        p0 = i * 64
                        # bufs=16: one slot per (h, qt) so this copy never
                        # carries a foreign slot-release wait
                        cU = rcpool.tile([DK + 1, 512], F32, tag="cU", bufs=16)
                        last_cU = cU
                        nc.vector.tensor_copy(cU[:], Cs[i][:])
                        rc = rcpool.tile([1, 512], F32, tag="rc")
                        nc.vector.reciprocal(rc[:], cU[DK : DK + 1, :])
                        Sb = rcpool.tile([DK, 512], F32, tag="Sbc")
                        nc.gpsimd.partition_broadcast(Sb[:], rc[:])
                        nc.vector.tensor_mul(
                            cT_sb[qt][p0 : p0 + 64, hp, :], cU[0:DK, :], Sb[:]
                        )

                # output projection for this qt; its full-array matmuls
                # interleave with the next qt's attention and keep the PE
                # activity monitor warm (partial; host sums head groups)
                for j in range(4):
                    t16 = qt * 4 + j
                    o_ps = psO.tile([128, D], F32, tag="o")
                    for nb in range(2):
                        for c2 in range(2):
                            nc.tensor.matmul(
                                o_ps[:, nb * 512 : (nb + 1) * 512],
                                lhsT=cT_sb[qt][:, c2, j * 128 : (j + 1) * 128],
                                rhs=wo_sb[:, c2, nb * 512 : (nb + 1) * 512],
                                start=(c2 == 0),
                                stop=(c2 == 1),
                            )
                    ob = osb.tile([128, D], F32, tag="ob")
                    nc.vector.tensor_copy(ob[:], o_ps[:])
                    nc.sync.dma_start(out_p[t16 * 128 : (t16 + 1) * 128, :], ob[:])


def _shard_inputs(query, key_in, value, Wq, bq, Wk, bk, Wv, bv, Wo, bo):
    q = np.ascontiguousarray(np.asarray(query, dtype=np.float32))
    k = np.ascontiguousarray(np.asarray(key_in, dtype=np.float32))
    v = np.ascontiguousarray(np.asarray(value, dtype=np.float32))
    Wq, Wk, Wv, Wo = (np.asarray(a, np.float32) for a in (Wq, Wk, Wv, Wo))
    bq, bk, bv = (np.asarray(a, np.float32) for a in (bq, bk, bv))

    in_maps = []
    for core in range(8):
        b, g = divmod(core, 4)
        sl = slice(g * HC, (g + 1) * HC)
        in_maps.append(
            {
                "xq_t": np.ascontiguousarray(q[b].T),
                "xk_t": np.ascontiguousarray(k[b].T),
                "xv_t": np.ascontiguousarray(v[b].T),
                "wq": np.ascontiguousarray(Wq[:, sl]),
                "wk": np.ascontiguousarray(Wk[:, sl]),
                "wv": np.ascontiguousarray(Wv[:, sl]),
                "wo": np.ascontiguousarray(Wo[sl, :]),
                "bq2": np.ascontiguousarray(bq[sl].reshape(2, 128).T),
                "bk2": np.ascontiguousarray(bk[sl].reshape(2, 128).T),
                "bv_bc": np.ascontiguousarray(
                    np.broadcast_to(bv[sl], (128, HC))
                ),
            }
        )
    return in_maps


def kernel(query, key_in, value, Wq, bq, Wk, bk, Wv, bv, Wo, bo):
    global LAST_RESULTS, _NC_CACHE
    if _NC_CACHE is None:
        _NC_CACHE = build_nc()
    nc = _NC_CACHE

    in_maps = _shard_inputs(query, key_in, value, Wq, bq, Wk, bk, Wv, bv, Wo, bo)
    trace = bool(os.environ.get("BASS_TRACE"))
    res = run_bass_kernel_spmd(nc, in_maps, core_ids=list(range(8)), trace=trace)
    LAST_RESULTS = res

    bo = np.asarray(bo, np.float32)
    out = np.empty((2, S, D), dtype=np.float32)
    for b in range(2):
        acc = res.results[4 * b]["out_p"].astype(np.float32)
        for g in range(1, 4):
            acc = acc + res.results[4 * b + g]["out_p"]
        out[b] = acc + bo
    return out
